# Initial kernel scaffold
#
"""Trainium2 Bass kernel for CPELayer_ResAG (concept-routed LoRA edit layer).

Computation (per token t with concept c = concept_idx[t]):
    down = edit_direction[t] @ lora_down[c]          # [768]@[768,4] -> [4]
    up   = down @ lora_up[c]                         # [4]@[4,1280]  -> [1280]
    out  = x[t] @ W.T + b_lin + 0.25 * up

Strategy: data-parallel over batch across 8 cores (616 tokens/core).
The routed LoRA is computed densely: A.T[(c,r), t] = lora_down_flat.T @ ed.T
for ALL concepts (only ~6% extra PE work), then masked on-device with a
one-hot built by DVE is_equal (the MoE routing), and contracted back with
lora_up_flat via the tensor engine, accumulating into the same PSUM as the
org matmul.  The bias is folded in as one extra contraction row (ones row in
the masked operand, b_lin row in the lora_up operand).  The 0.25 LoRA scale
is folded into lora_up host-side (exact: power of two).

All host-side work is layout only (transpose / reshape / concat / dtype of
the int indices to f32); every FLOP of the reference runs on device.
"""

import sys
import types

import numpy as np

import concourse.mybir as mybir
import concourse.tile as tile
from concourse import bacc
from concourse.bass_utils import run_bass_kernel_spmd

# If BASS_TRACE is set in the environment, run_bass_kernel_spmd imports
# antenv.axon_hooks, which some containers lack; stub it (None hook ->
# tracing is skipped gracefully, execution unaffected).
try:
    import antenv.axon_hooks  # noqa: F401
except ImportError:
    _m = types.ModuleType("antenv.axon_hooks")
    _m.get_axon_ntff_profile_hook = lambda: None
    _m.set_axon_ntff_profile_hook = lambda h: None
    sys.modules["antenv.axon_hooks"] = _m

# Problem shapes (hardcoded per spec nn_CPELayer_ResAG_19335942766951)
N_CORES = 8
B, T, DIN, DOUT = 64, 77, 768, 1280
N_CONCEPTS, RANK = 50, 4
SCALE = 0.25  # alpha/rank = 1/4, exact power of two
BPC = B // N_CORES          # batches per core = 8
TOK = BPC * T               # tokens per core = 616
NJ = N_CONCEPTS * RANK      # 200 flattened (concept, rank) rows
KJ_PAD = 256                # padded rows: 200 lora + 1 bias + 55 zero
P = 128
KD = DIN // P               # 6 k-tiles of the d_in contraction
NH = 308                    # half of TOK for the A.T psum tiles (>=256 keeps f32r full-rate)
T_EDGES = [0, 128, 256, 384, 512, 616]
N_CHUNKS = [(0, 512), (512, 512), (1024, 256)]

_cache = {}


def _build_bass(mm_dtype, lora_dtype=None):
    nc = bacc.Bacc("TRN2", target_bir_lowering=False, debug=False,
                   num_devices=N_CORES)
    f32 = mybir.dt.float32
    # Tensors consumed by the tensor engine carry the matmul dtype end-to-end
    # (float32r is fp32-layout; the BIR verifier requires producer outputs to
    # be fp32r-typed when a fp32r matmul consumes them).  The LoRA branch
    # (edT/ldT/luB/MT) contributes only ~0.7% of the output scale, so it can
    # run at a lower precision than the org matmul without moving the
    # end-to-end error.
    sdt = mm_dtype
    ldt = lora_dtype if lora_dtype is not None else mm_dtype

    xT_d = nc.dram_tensor("xT", [DIN, TOK], sdt, kind="ExternalInput").ap()
    edT_d = nc.dram_tensor("edT", [DIN, TOK], ldt, kind="ExternalInput").ap()
    idx_d = nc.dram_tensor("idxf", [1, TOK], f32, kind="ExternalInput").ap()
    cv_d = nc.dram_tensor("cvals", [P, 2], f32, kind="ExternalInput").ap()
    WT_d = nc.dram_tensor("WT", [DIN, DOUT], sdt, kind="ExternalInput").ap()
    ldT_d = nc.dram_tensor("ldT", [DIN, NJ], ldt, kind="ExternalInput").ap()
    lu_d = nc.dram_tensor("luB", [KJ_PAD, DOUT], ldt, kind="ExternalInput").ap()
    out_d = nc.dram_tensor("out", [TOK, DOUT], f32, kind="ExternalOutput").ap()

    with tile.TileContext(nc) as tc:
        with (
            tc.tile_pool(name="consts", bufs=1) as consts,
            tc.tile_pool(name="outsb", bufs=5) as outsb,
        ):
            # Load order matters: tiny routing tensors first (masks unblock),
            # then ldT/edT (the A.T chain), then luB (up-matmul rhs) so the
            # early wave-A matmuls can run, and the bulky org operands
            # (xT/WT) last, streaming k-pair by k-pair with the org matmuls
            # tracking their arrival.
            cvals = consts.tile([P, 2], f32, tag="cvals")
            nc.sync.dma_start(cvals[:], cv_d[:, :])

            xT = [None] * KD
            WT = [None] * KD

            def load_kpair(k):
                t_ = consts.tile([P, TOK], sdt, tag=f"xT{k}")
                nc.sync.dma_start(t_[:], xT_d[k * P:(k + 1) * P, :])
                xT[k] = t_
                t_ = consts.tile([P, DOUT], sdt, tag=f"WT{k}")
                nc.sync.dma_start(t_[:], WT_d[k * P:(k + 1) * P, :])
                WT[k] = t_

            # The LoRA-side tensors load as ONE DMA each (3D access pattern,
            # k-tiles side by side in the free dim): 3 sequencer issues
            # instead of 14, so the A.T/MT critical path unblocks ~4us
            # earlier (the small loads were issue-rate bound, not
            # bandwidth bound).
            ld_all = consts.tile([P, KD, NJ], ldt, tag="ld_all")
            nc.sync.dma_start(ld_all[:],
                              ldT_d.rearrange("(k p) j -> p k j", p=P))
            # ed in two halves: the A.T matmuls on k0..2 start while k3..5
            # is still in flight (PE end time = start + busy, so an earlier
            # start is a direct win).
            KH = KD // 2
            ed_a = consts.tile([P, KH, TOK], ldt, tag="ed_a")
            nc.sync.dma_start(ed_a[:],
                              edT_d[0:KH * P, :].rearrange(
                                  "(k p) t -> p k t", p=P))
            ed_b = consts.tile([P, KD - KH, TOK], ldt, tag="ed_b")
            nc.sync.dma_start(ed_b[:],
                              edT_d[KH * P:DIN, :].rearrange(
                                  "(k p) t -> p k t", p=P))
            # Broadcast the token->concept ids across all 128 partitions so a
            # per-partition-scalar is_equal against cvals builds the one-hot.
            idx_bc = consts.tile([P, TOK], f32, tag="idx_bc")
            nc.sync.dma_start(idx_bc[:], idx_d.partition_broadcast(P))

            lu_all = consts.tile([P, 2, DOUT], ldt, tag="lu_all")
            nc.sync.dma_start(lu_all[:],
                              lu_d.rearrange("(j p) o -> p j o", p=P))
            ldT = [ld_all[:, k, :] for k in range(KD)]
            edT = ([ed_a[:, k, :] for k in range(KH)]
                   + [ed_b[:, k, :] for k in range(KD - KH)])
            lu = [lu_all[:, j, :] for j in range(2)]
            for k in range(KD):
                load_kpair(k)

            masks = []
            for jc in range(2):
                m = consts.tile([P, TOK], f32, tag=f"mask{jc}")
                nc.vector.tensor_scalar(
                    m[:], idx_bc[:], cvals[:, jc:jc + 1], None,
                    mybir.AluOpType.is_equal)
                masks.append(m)

            # A.T[(c,r), t] = lora_down_flat.T @ ed.T  for all concepts,
            # masked into MT (the routed "down" activations, transposed).
            MT = []
            for jc in range(2):
                t_ = consts.tile([P, TOK], ldt, tag=f"MT{jc}")
                MT.append(t_)
            # Chunk-1 rows 72..127 pair with luB rows 200..255: engine ops
            # need a 32-aligned start partition, so zero 64..128 first, then
            # the ones row at 96 (bias: b_lin sits at luB[224]); the mask-mul
            # below overwrites rows 0..71 (lora j=128..199).
            # (memset can't target float32r; synthesize 0s/1s via DVE with
            # idx_bc as a donor input, converted on write)
            nc.vector.tensor_scalar(
                MT[1][64:P, :], idx_bc[64:P, :], 0.0, None,
                mybir.AluOpType.mult)
            nc.vector.tensor_scalar(
                MT[1][96:97, :], idx_bc[96:97, :], 0.0, 1.0,
                mybir.AluOpType.mult, mybir.AluOpType.add)

            with tc.tile_pool(name="at_ps", bufs=4, space="PSUM") as at_pool:
                for jc in range(2):
                    jp = P if jc == 0 else NJ - P  # 128, 72
                    jsl = slice(jc * P, jc * P + jp)
                    for nh in range(2):
                        nsl = slice(nh * NH, (nh + 1) * NH)
                        at = at_pool.tile([P, NH], f32, tag="at")
                        for k in range(KD):
                            nc.tensor.matmul(
                                at[:jp, :], ldT[k][:, jsl], edT[k][:, nsl],
                                start=(k == 0), stop=(k == KD - 1))
                        nc.vector.tensor_tensor(
                            MT[jc][:jp, nsl], at[:jp, :], masks[jc][:jp, nsl],
                            mybir.AluOpType.mult)

            # Main accumulation, two short-lived PSUM waves per (t, n) so
            # banks recycle during the load phase instead of every group
            # staying open until the last WT k-tile arrives:
            #   wave A: up1+up2 (MT/lu ready early) + org k0..k2 -> copy osb
            #   wave B: org k3..k5 -> DVE-add into osb
            KA = 3  # org k-tiles in wave A
            with tc.tile_pool(name="out_ps", bufs=8, space="PSUM") as out_pool:
                osbs = []
                for ti in range(len(T_EDGES) - 1):
                    t0, t1 = T_EDGES[ti], T_EDGES[ti + 1]
                    tw = t1 - t0
                    tsl = slice(t0, t1)
                    osb = outsb.tile([P, DOUT], f32, tag="osb")
                    osbs.append(osb)
                    for (n0, nw) in N_CHUNKS:
                        ps = out_pool.tile([P, 512], f32, tag="ops")
                        nmm = 2 + KA
                        i = 0
                        for jc in range(2):
                            nc.tensor.matmul(
                                ps[:tw, :nw], MT[jc][:, tsl],
                                lu[jc][:, n0:n0 + nw],
                                start=(i == 0), stop=(i == nmm - 1))
                            i += 1
                        for k in range(KA):
                            nc.tensor.matmul(
                                ps[:tw, :nw], xT[k][:, tsl],
                                WT[k][:, n0:n0 + nw],
                                start=(i == 0), stop=(i == nmm - 1))
                            i += 1
                        nc.any.tensor_copy(out=osb[:tw, n0:n0 + nw],
                                           in_=ps[:tw, :nw])
                for ti in range(len(T_EDGES) - 1):
                    t0, t1 = T_EDGES[ti], T_EDGES[ti + 1]
                    tw = t1 - t0
                    tsl = slice(t0, t1)
                    osb = osbs[ti]
                    for (n0, nw) in N_CHUNKS:
                        ps = out_pool.tile([P, 512], f32, tag="ops")
                        for i, k in enumerate(range(KA, KD)):
                            nc.tensor.matmul(
                                ps[:tw, :nw], xT[k][:, tsl],
                                WT[k][:, n0:n0 + nw],
                                start=(i == 0), stop=(i == KD - KA - 1))
                        nc.vector.tensor_tensor(
                            osb[:tw, n0:n0 + nw], ps[:tw, :nw],
                            osb[:tw, n0:n0 + nw], mybir.AluOpType.add)
                    nc.sync.dma_start(out_d[tsl, :], osb[:tw, :])

    nc.compile()
    return nc


def get_bass(mm_dtype=None, lora_dtype=None):
    if mm_dtype is None:
        mm_dtype = mybir.dt.float32r
        if lora_dtype is None:
            lora_dtype = mybir.dt.bfloat16
    if lora_dtype is None:
        lora_dtype = mm_dtype
    key = (str(mm_dtype), str(lora_dtype))
    if key not in _cache:
        _cache[key] = _build_bass(mm_dtype, lora_dtype)
    return _cache[key]


def make_in_maps(x, edit_direction, concept_idx, lora_down, lora_up, W, b_lin,
                 np_sdt=np.float32, np_ldt=None):
    """Host-side sharding + layout prep (no reference FLOPs).

    np_sdt: numpy dtype for the org-matmul tensors (xT/WT); np_ldt: dtype
    for the LoRA-branch tensors (edT/ldT/luB), defaults to np_sdt."""
    if np_ldt is None:
        np_ldt = np_sdt
    x = np.asarray(x, dtype=np.float32)
    ed = np.asarray(edit_direction, dtype=np.float32)
    idx = np.asarray(concept_idx)
    ld = np.asarray(lora_down, dtype=np.float32)
    lup = np.asarray(lora_up, dtype=np.float32)
    W = np.asarray(W, dtype=np.float32)
    b = np.asarray(b_lin, dtype=np.float32)

    WT = np.ascontiguousarray(W.T.astype(np_sdt))               # [768, 1280]
    ldT = np.ascontiguousarray(
        ld.transpose(1, 0, 2).reshape(DIN, NJ).astype(np_ldt))
    luB = np.zeros((KJ_PAD, DOUT), dtype=np.float32)
    luB[:NJ] = lup.reshape(NJ, DOUT) * SCALE                    # exact x0.25
    luB[128 + 96] = b                                           # bias row
    luB = luB.astype(np_ldt)
    cv = np.full(2 * P, -1.0, dtype=np.float32)
    cv[:NJ] = np.arange(NJ, dtype=np.float32) // RANK
    cvals = np.ascontiguousarray(cv.reshape(2, P).T)            # [128, 2]

    in_maps = []
    for c in range(N_CORES):
        sl = slice(c * BPC, (c + 1) * BPC)
        xs = x[sl].reshape(TOK, DIN)
        eds = ed[sl].reshape(TOK, DIN)
        idxs = idx[sl].reshape(TOK).astype(np.float32)
        in_maps.append({
            "xT": np.ascontiguousarray(xs.T.astype(np_sdt)),
            "edT": np.ascontiguousarray(eds.T.astype(np_ldt)),
            "idxf": np.ascontiguousarray(idxs.reshape(1, TOK)),
            "cvals": cvals,
            "WT": WT,
            "ldT": ldT,
            "luB": luB,
        })
    return in_maps


def kernel(x, edit_direction, concept_idx, lora_down, lora_up, W, b_lin,
           _trace=False, _mm_dtype=None, _lora_dtype=None):
    if _mm_dtype is None:
        _mm_dtype = mybir.dt.float32r
        if _lora_dtype is None:
            _lora_dtype = mybir.dt.bfloat16
    if _lora_dtype is None:
        _lora_dtype = _mm_dtype
    nc = get_bass(_mm_dtype, _lora_dtype)
    in_maps = make_in_maps(x, edit_direction, concept_idx, lora_down, lora_up,
                           W, b_lin, np_sdt=mybir.dt.np(_mm_dtype),
                           np_ldt=mybir.dt.np(_lora_dtype))
    res = run_bass_kernel_spmd(nc, in_maps, core_ids=list(range(N_CORES)),
                               trace=_trace)
    out = np.concatenate([r["out"] for r in res.results], axis=0)
    out = out.reshape(B, T, DOUT)
    if _trace:
        kernel.last_results = res
    return out



# revision 7
# speedup vs baseline: 1.2152x; 1.2152x over previous
"""Trainium2 Bass kernel for CPELayer_ResAG (concept-routed LoRA edit layer).

Computation (per token t with concept c = concept_idx[t]):
    down = edit_direction[t] @ lora_down[c]          # [768]@[768,4] -> [4]
    up   = down @ lora_up[c]                         # [4]@[4,1280]  -> [1280]
    out  = x[t] @ W.T + b_lin + 0.25 * up

Strategy: data-parallel over batch across 8 cores (616 tokens/core).
The routed LoRA is computed densely for ALL concepts (A.T = ld.T @ ed.T,
~6% extra PE work), masked on-device with a one-hot built by DVE is_equal,
and contracted back with lora_up, accumulating into the same PSUM group as
the org matmul.

v2 schedule (vs v1): LoRA-branch matmuls run in fp8-e4m3 with
perf_mode=DoubleRow (2 contraction rows per PE cell: operands are 3D
[128, 2, N] tiles, contraction row = subtile*128 + partition), halving
their PE column counts.  All power-of-two scale corrections (ld x16 on
the host, x1/128 in the mask value, lu x2 / bias x8 on the host) keep the
math exact.  Inputs arrive in 8 consolidated DMAs issued from three
different engine queues (sync/scalar) so doorbell issue doesn't serialize
ahead of the first matmul; outputs leave as bf16 (upconverted on the
host) from the gpsimd queue.  The org matmul streams as two waves per
(t, n) PSUM group - wave1 [upDR, k0..k2] -> copy to SBUF, wave2 [k3..k5]
-> DVE add - so PE work overlaps the staged xT/WT arrival.

Host-side work is layout/dtype only; every FLOP of the reference runs on
device.
"""

import sys
import types

import numpy as np

import concourse.mybir as mybir
import concourse.tile as tile
from concourse import bacc
from concourse.bass_utils import run_bass_kernel_spmd

# If BASS_TRACE is set in the environment, run_bass_kernel_spmd imports
# antenv.axon_hooks, which some containers lack; stub it (None hook ->
# tracing is skipped gracefully, execution unaffected).
try:
    import antenv.axon_hooks  # noqa: F401
except ImportError:
    _m = types.ModuleType("antenv.axon_hooks")
    _m.get_axon_ntff_profile_hook = lambda: None
    _m.set_axon_ntff_profile_hook = lambda h: None
    sys.modules["antenv.axon_hooks"] = _m

# Problem shapes (hardcoded per spec nn_CPELayer_ResAG_19335942766951)
N_CORES = 8
B, T, DIN, DOUT = 64, 77, 768, 1280
N_CONCEPTS, RANK = 50, 4
BPC = B // N_CORES          # batches per core = 8
TOK = BPC * T               # tokens per core = 616
NJ = N_CONCEPTS * RANK      # 200 flattened (concept, rank) rows
P = 128
KD = DIN // P               # 6 k-tiles of the d_in contraction
NH = 308                    # half of TOK for the A.T psum tiles
# led layout: [:, 0:200] = ldT*16, the two 308-col halves of edT at
# 16B-aligned offsets 208 and 528 (DoubleRow requires 16-aligned bases and
# subtile steps; LED_W=848 keeps the subtile byte-step 16-aligned too).
ED_OFFS = (208, 528)
LED_W = 848
XW_W = TOK + DOUT           # 1896: [:, 0:616]=xT k-tile, [:, 616:]=WT k-tile
T_EDGES = [0, 128, 256, 384, 512, 616]
N_CHUNKS = [(0, 512), (512, 512), (1024, 256)]

_cache = {}


def _build_bass():
    nc = bacc.Bacc("TRN2", target_bir_lowering=False, debug=False,
                   num_devices=N_CORES)
    f32 = mybir.dt.float32
    bf16 = mybir.dt.bfloat16
    f8 = mybir.dt.float8e4
    DR = mybir.MatmulPerfMode.DoubleRow

    led_d = nc.dram_tensor("led", [DIN, LED_W], f8, kind="ExternalInput").ap()
    lu_d = nc.dram_tensor("lu8", [2 * P, DOUT], f8, kind="ExternalInput").ap()
    idx_d = nc.dram_tensor("idxf", [1, TOK], f32, kind="ExternalInput").ap()
    cv_d = nc.dram_tensor("cvals", [P, 2], f32, kind="ExternalInput").ap()
    xw_d = nc.dram_tensor("xw", [DIN, XW_W], bf16, kind="ExternalInput").ap()
    out_d = nc.dram_tensor("out", [TOK, DOUT], bf16, kind="ExternalOutput").ap()

    with tile.TileContext(nc) as tc:
        with (
            tc.tile_pool(name="consts", bufs=1) as consts,
            tc.tile_pool(name="outsb", bufs=5) as outsb,
        ):
            # Input DMAs: the sync queue carries the big data stream in
            # arrival-priority order; the tiny routing tensors ride the
            # scalar queue so their doorbells don't delay the stream.
            led_a = consts.tile([P, 2, LED_W], f8, tag="led_a")
            nc.sync.dma_start(led_a[:],
                              led_d[0:2 * P, :].rearrange(
                                  "(k p) c -> p k c", p=P))
            led_b = consts.tile([P, KD - 2, LED_W], f8, tag="led_b")
            nc.sync.dma_start(led_b[:],
                              led_d[2 * P:DIN, :].rearrange(
                                  "(k p) c -> p k c", p=P))
            xw0 = consts.tile([P, 1, XW_W], bf16, tag="xw0")
            nc.sync.dma_start(xw0[:],
                              xw_d[0:P, :].rearrange("(k p) c -> p k c", p=P))
            xw12 = consts.tile([P, 2, XW_W], bf16, tag="xw12")
            nc.sync.dma_start(xw12[:],
                              xw_d[P:3 * P, :].rearrange(
                                  "(k p) c -> p k c", p=P))
            xw345 = consts.tile([P, 3, XW_W], bf16, tag="xw345")
            nc.sync.dma_start(xw345[:],
                              xw_d[3 * P:DIN, :].rearrange(
                                  "(k p) c -> p k c", p=P))

            cvals = consts.tile([P, 2], f32, tag="cvals")
            nc.scalar.dma_start(cvals[:], cv_d[:, :])
            idx_bc = consts.tile([P, TOK], f32, tag="idx_bc")
            nc.scalar.dma_start(idx_bc[:], idx_d.partition_broadcast(P))
            lu8 = consts.tile([P, 2, DOUT], f8, tag="lu8")
            nc.scalar.dma_start(lu8[:],
                                lu_d.rearrange("(j p) o -> p j o", p=P))

            def led_pair(kk, csl):  # 3D [128, 2, csl] DoubleRow operand
                t_ = led_a if kk < 2 else led_b
                o = 0 if kk < 2 else 2
                return t_[:, kk - o:kk - o + 2, csl]

            def xt(kk, tsl):
                t_, o = ((xw0, 0) if kk < 1 else
                         (xw12, 1) if kk < 3 else (xw345, 3))
                return t_[:, kk - o, tsl]

            def wt(kk, nsl):
                t_, o = ((xw0, 0) if kk < 1 else
                         (xw12, 1) if kk < 3 else (xw345, 3))
                return t_[:, kk - o, slice(TOK + nsl.start, TOK + nsl.stop)]

            # One-hot masks: mask[p, t] = (concept_idx[t] == cvals[p, jc])
            # scaled by 2^-7 (folds away the host-side ld x16 and the fp8
            # dynamic-range shift s=8 on MT8).
            masks = []
            for jc in range(2):
                m = consts.tile([P, TOK], f32, tag=f"mask{jc}")
                nc.vector.tensor_scalar(
                    m[:], idx_bc[:], cvals[:, jc:jc + 1], 1.0 / 128.0,
                    mybir.AluOpType.is_equal, mybir.AluOpType.mult)
                masks.append(m)

            # MT8[(c,r) rows as [part, subtile], t]: routed "down" activations
            # in fp8, consumed by the DoubleRow up-matmul.  Rows 200..255
            # (partitions 72..127 of subtile 1) are zero except the ones row
            # at 224 (partition 96) that contracts with the bias row of lu8.
            MT8 = consts.tile([P, 2, 640], f8, tag="MT8")
            nc.gpsimd.memset(MT8[64:P, 1, :], 0.0)
            nc.gpsimd.memset(MT8[96:97, 1, :], 0.125)

            # A.T[(c,r), t] = (16*lora_down_flat).T @ ed.T for all concepts,
            # 3 DoubleRow matmuls per chunk (contraction row = sub*128+p).
            with tc.tile_pool(name="at_ps", bufs=4, space="PSUM") as at_pool:
                for jc in range(2):
                    jp = P if jc == 0 else NJ - P  # 128, 72
                    jsl = slice(jc * P, jc * P + jp)
                    for nh in range(2):
                        nsl = slice(nh * NH, (nh + 1) * NH)
                        esl = slice(ED_OFFS[nh], ED_OFFS[nh] + NH)
                        at = at_pool.tile([P, NH], f32, tag="at")
                        for k in range(KD // 2):
                            nc.tensor.matmul(
                                at[:jp, :], led_pair(2 * k, jsl),
                                led_pair(2 * k, esl),
                                start=(k == 0), stop=(k == KD // 2 - 1),
                                perf_mode=DR)
                        nc.vector.tensor_tensor(
                            MT8[:jp, jc, nsl], at[:jp, :],
                            masks[jc][:jp, nsl], mybir.AluOpType.mult)

            # Main accumulation: wave1 [upDR, org k0..k2] per (t, n) PSUM
            # group -> copy to bf16 osb (alternating scalar/vector); wave2
            # [org k3..k5] -> vector add into osb -> bf16 output DMA from
            # the gpsimd queue.
            KA = 3
            with tc.tile_pool(name="out_ps", bufs=8, space="PSUM") as out_pool:
                osbs = []
                ei = 0
                for ti in range(len(T_EDGES) - 1):
                    t0, t1 = T_EDGES[ti], T_EDGES[ti + 1]
                    tw = t1 - t0
                    tsl = slice(t0, t1)
                    osb = outsb.tile([P, DOUT], bf16, tag="osb")
                    osbs.append(osb)
                    for (n0, nw) in N_CHUNKS:
                        nsl = slice(n0, n0 + nw)
                        ps = out_pool.tile([P, 512], f32, tag="ops")
                        nc.tensor.matmul(
                            ps[:tw, :nw], MT8[:, 0:2, tsl],
                            lu8[:, 0:2, nsl],
                            start=True, stop=False, perf_mode=DR)
                        for k in range(KA):
                            nc.tensor.matmul(
                                ps[:tw, :nw], xt(k, tsl), wt(k, nsl),
                                start=False, stop=(k == KA - 1))
                        if ei % 2 == 0:
                            nc.scalar.copy(out=osb[:tw, nsl],
                                           in_=ps[:tw, :nw])
                        else:
                            nc.vector.tensor_copy(out=osb[:tw, nsl],
                                                  in_=ps[:tw, :nw])
                        ei += 1
                for ti in range(len(T_EDGES) - 1):
                    t0, t1 = T_EDGES[ti], T_EDGES[ti + 1]
                    tw = t1 - t0
                    tsl = slice(t0, t1)
                    osb = osbs[ti]
                    for (n0, nw) in N_CHUNKS:
                        nsl = slice(n0, n0 + nw)
                        ps = out_pool.tile([P, 512], f32, tag="ops")
                        for i, k in enumerate(range(KA, KD)):
                            nc.tensor.matmul(
                                ps[:tw, :nw], xt(k, tsl), wt(k, nsl),
                                start=(i == 0), stop=(i == KD - KA - 1))
                        nc.vector.tensor_tensor(
                            osb[:tw, nsl], ps[:tw, :nw], osb[:tw, nsl],
                            mybir.AluOpType.add)
                    nc.gpsimd.dma_start(out_d[tsl, :], osb[:tw, :])

    nc.compile()
    return nc


def get_bass():
    if "v2" not in _cache:
        _cache["v2"] = _build_bass()
    return _cache["v2"]


def make_in_maps(x, edit_direction, concept_idx, lora_down, lora_up, W, b_lin):
    """Host-side sharding + layout/dtype prep (no reference FLOPs)."""
    f8 = mybir.dt.np(mybir.dt.float8e4)
    bf = mybir.dt.np(mybir.dt.bfloat16)
    x = np.asarray(x, dtype=np.float32)
    ed = np.asarray(edit_direction, dtype=np.float32)
    idx = np.asarray(concept_idx)
    ld = np.asarray(lora_down, dtype=np.float32)
    lup = np.asarray(lora_up, dtype=np.float32)
    W = np.asarray(W, dtype=np.float32)
    b = np.asarray(b_lin, dtype=np.float32)

    ldT = ld.transpose(1, 0, 2).reshape(DIN, NJ)                # [768, 200]
    lu8 = np.zeros((2 * P, DOUT), dtype=np.float32)
    lu8[:NJ] = lup.reshape(NJ, DOUT) * 2.0   # x8 (range) x0.25 (alpha/rank)
    lu8[P + 96] = b * 8.0                    # bias row (ones row is 1/8)
    lu8 = lu8.astype(f8)
    cv = np.full(2 * P, -1.0, dtype=np.float32)
    cv[:NJ] = np.arange(NJ, dtype=np.float32) // RANK
    cvals = np.ascontiguousarray(cv.reshape(2, P).T)            # [128, 2]
    WT = W.T.astype(bf)                                         # [768, 1280]

    in_maps = []
    for c in range(N_CORES):
        sl = slice(c * BPC, (c + 1) * BPC)
        xs = x[sl].reshape(TOK, DIN)
        eds = ed[sl].reshape(TOK, DIN)
        idxs = idx[sl].reshape(TOK).astype(np.float32)
        led = np.zeros((DIN, LED_W), dtype=f8)
        led[:, :NJ] = (ldT * 16.0).astype(f8)
        edT8 = eds.T.astype(f8)
        led[:, ED_OFFS[0]:ED_OFFS[0] + NH] = edT8[:, :NH]
        led[:, ED_OFFS[1]:ED_OFFS[1] + NH] = edT8[:, NH:]
        xw = np.empty((DIN, XW_W), dtype=bf)
        xw[:, :TOK] = xs.T.astype(bf)
        xw[:, TOK:] = WT
        in_maps.append({
            "led": led,
            "lu8": lu8,
            "idxf": np.ascontiguousarray(idxs.reshape(1, TOK)),
            "cvals": cvals,
            "xw": xw,
        })
    return in_maps


def kernel(x, edit_direction, concept_idx, lora_down, lora_up, W, b_lin,
           _trace=False):
    nc = get_bass()
    in_maps = make_in_maps(x, edit_direction, concept_idx, lora_down, lora_up,
                           W, b_lin)
    res = run_bass_kernel_spmd(nc, in_maps, core_ids=list(range(N_CORES)),
                               trace=_trace)
    out = np.concatenate([np.asarray(r["out"], dtype=np.float32)
                          for r in res.results], axis=0)
    out = out.reshape(B, T, DOUT)
    if _trace:
        kernel.last_results = res
    return out
